# revision 45
# baseline (speedup 1.0000x reference)
"""Multi-head attention (b=2, n=2048, dim=1024, h=16, dh=64) on 8 TRN2 NeuronCores.

Sharding: 32 (batch, head) pairs -> 8 cores x (1 batch, 4 heads). No collectives.
Per core:
  inputs : xT  [128, 8*2048] bf16 (x[b].T packed n-chunk-major: element
                                   (p, nch, kt, n) = x[b].T[kt*128+p, nch*512+n])
           wq  [1024, 256]  bf16  (q-columns of w_qkv for this core's 4 heads, pre-scaled by 1/8)
           wk  [1024, 256]  bf16
           wv  [1024, 256]  bf16
  output : out [4*65, 2048] f32   (per local head: rows 0-63 = unnormalized (attn@v)^T,
                                   row 64 = softmax denominator per query)
Host divides by the denominator and transposes back to [b, n, h*dh].

Device pipeline per core:
  qT/kT = (w.T @ x.T) in [d, n] layout, head-pairs packed 2x64 on partitions (bf16)
  V     = (x @ wv)    in [n, d] layout with a ones column appended (bf16)
  per head pair, per 512-wide query chunk, per 128-wide key block:
    S^T[j,i] = kT.T @ qT   (two K=64 matmuls packed into PE row-groups 0-63 / 64-127)
    A^T      = exp(S^T)    (ACT f32->bf16 for most key blocks; for jb in S_DVE the
                            DVE computes a Schraudolph bit-hack exp instead:
                            bf16_bits = round(s*128*log2e + 16256 - C) as int16,
                            consumed by the PV matmul via a bf16 bitcast view)
    O^T     += [V|1].T @ A^T  (PSUM-accumulated over key blocks; row 64 = rowsum)

The xT DMA is n-chunk-major so the first projection (and hence the exp stream on
the critical ACT engine) starts after 1/4 of the x transfer instead of all of it.
"""

import numpy as np
import ml_dtypes

B, N, DIM = 2, 2048, 1024
HEADS, DH = 16, 64
P = 128
KT = DIM // P          # 8 k-tiles
NT = N // P            # 16 n/j blocks
NCH = N // 512         # 4 chunks of 512
HL = 4                 # local heads per core
OROWS = HL * (DH + 1)  # 260 output rows per core

# Schraudolph fast-exp constants (bf16 bit hack on the DVE engine):
#   bits = s * 128/ln(2) + (16256 - C [+0.5 for truncating converts])
# C ~= 7.33 zeroes the mean log-error so DVE-offloaded key blocks are
# unbiased relative to ACT-computed ones (the residual is a ~2% sawtooth
# that partially cancels in the softmax normalization).
SCH_A = 184.6650308540
SCH_C = 7.33
SCH_B = 16256.0 - SCH_C + 0.5
# Key blocks whose exp runs on the DVE, per attention block.  Blocks 0-3 are
# PE-bound (projection weave + deferred PV), so all their exps stay on ACT,
# which also keeps the projection-copy weave (a DVE op) free of same-queue
# deadlocks there; blocks 4-7 are exp-paced and offload 5/16 tiles.
S_DVE_EARLY = ()
S_DVE_MAIN = (2, 5, 8, 11, 14)


def _s_dve(b):
    return S_DVE_EARLY if b <= 3 else S_DVE_MAIN

_CACHE = {}
LAST_RESULTS = None
TRACE = False


def _build_nc():
    from contextlib import ExitStack

    import concourse.bass as bass
    import concourse.tile as tile
    from concourse import bacc, mybir

    bf16 = mybir.dt.bfloat16
    i16 = mybir.dt.int16
    f32 = mybir.dt.float32

    nc = bacc.Bacc("TRN2", target_bir_lowering=False)

    xT_d = nc.dram_tensor("xT", [P, KT * N], bf16, kind="ExternalInput")
    wq_d = nc.dram_tensor("wq", [DIM, HL * DH], bf16, kind="ExternalInput")
    wk_d = nc.dram_tensor("wk", [DIM, HL * DH], bf16, kind="ExternalInput")
    wv_d = nc.dram_tensor("wv", [DIM, HL * DH], bf16, kind="ExternalInput")
    out_d = nc.dram_tensor("out", [OROWS, N], f32, kind="ExternalOutput")

    # out rows viewed as [row-within-head, head, n] for packed output DMAs
    out_r = out_d[:, :].rearrange("(hh r) n -> r hh n", r=DH + 1)
    wq_r = wq_d[:, :].rearrange("(kt p) c -> p kt c", p=P)
    wk_r = wk_d[:, :].rearrange("(kt p) c -> p kt c", p=P)
    wv_r = wv_d[:, :].rearrange("(kt p) c -> p kt c", p=P)

    with tile.TileContext(nc) as tc, ExitStack() as ctx:
        sing = ctx.enter_context(tc.tile_pool(name="sing", bufs=1))
        spool = ctx.enter_context(
            tc.tile_pool(name="s_ps", bufs=3, space=bass.MemorySpace.PSUM)
        )
        opool = ctx.enter_context(
            tc.tile_pool(name="o_ps", bufs=1, space=bass.MemorySpace.PSUM)
        )
        apool = ctx.enter_context(tc.tile_pool(name="a_sb", bufs=28))
        copool = ctx.enter_context(tc.tile_pool(name="o_sb", bufs=4))

        # persistent SBUF tensors; xT is n-chunk-major: [p, nch, kt, n]
        xT = sing.tile([P, NCH, KT, 512], bf16, tag="xT")
        wq = sing.tile([P, KT, HL * DH], bf16, tag="wq")
        wk = sing.tile([P, KT, HL * DH], bf16, tag="wk")
        wv = sing.tile([P, KT, HL * DH], bf16, tag="wv")
        # head-pair packed projections: partitions 0-63 head A dims, 64-127 head B
        qT = [sing.tile([P, N], bf16, tag=f"qT{i}", name=f"qT{i}") for i in range(2)]
        kT = [sing.tile([P, N], bf16, tag=f"kT{i}", name=f"kT{i}") for i in range(2)]
        # V in [j, d] layout per j-block per head, with ones column at d=64
        v = sing.tile([P, NT, HL, DH + 1], bf16, tag="v")

        # input DMAs: STRICTLY SERIAL on one HWDGE ring in dependency order
        # (wk, x-chunk0, wq, then the rest).  Splitting across rings makes the
        # transfers share HBM bandwidth round-robin, so chunk 0 — which gates
        # the first projection and hence the whole exp stream — would finish
        # last instead of first (measured: parallel splits cost 10-40us).
        # wv rides the software DGE; it isn't needed until the V weave.
        nc.gpsimd.dma_start(out=wv[:], in_=wv_r[:])
        xT_f = xT[:].rearrange("p c kt n -> p (c kt n)")
        # kt-half splits (contiguous rows) let the first half of the k
        # projection overlap the tail of the chunk-0 transfer
        nc.sync.dma_start(out=wk[:, 0:4, :], in_=wk_r[:, 0:4, :])
        nc.sync.dma_start(out=xT_f[:, 0:2048], in_=xT_d[:, 0:2048])
        nc.sync.dma_start(out=wq[:], in_=wq_r[:])
        nc.sync.dma_start(out=wk[:, 4:8, :], in_=wk_r[:, 4:8, :])
        nc.sync.dma_start(out=xT_f[:, 2048:4096], in_=xT_d[:, 2048:4096])
        for c in (1, 2, 3):
            nc.sync.dma_start(
                out=xT_f[:, c * 4096 : (c + 1) * 4096],
                in_=xT_d[:, c * 4096 : (c + 1) * 4096],
            )

        # ---- spool slot-consumer tracking ----
        # Every spool.tile() allocation is logged with who consumes the tile.
        # A projection/V copy runs on the DVE; its matmuls wait for the slot
        # (3 allocations back) to be freed by that slot's consumer.  If that
        # consumer were a DVE Schraudolph emitted LATER than the copy, the DVE
        # queue would deadlock on itself, so the weave only emits a unit when
        # the slot's pending consumer is an ACT exp or an already-emitted DVE
        # op.  Consumers are tagged (kind, period).
        sp_log = []

        def sp_alloc(shape, consumer):
            sp_log.append(consumer)
            return spool.tile(shape, f32, tag="sp", name="sp")

        def weave_safe(cur_period):
            if len(sp_log) < 3:
                return True
            kind, period = sp_log[-3]
            return kind != "schr" or period <= cur_period

        # ---- projections ----
        done = set()  # emitted projection/V units, for deadline asserts

        # k, q: out[c, n] = w[:, c].T @ xT, one 512-col chunk at a time.
        def proj_unit(wt, dst, hp, nch, key):
            """Emit the 8 K-accumulated matmuls + copy for one 512-col chunk."""

            def work(cur_period):
                ps = sp_alloc([P, 512], ("copy", cur_period))
                for kt in range(KT):
                    nc.tensor.matmul(
                        ps[:],
                        wt[:, kt, hp * P : (hp + 1) * P],
                        xT[:, nch, kt, :],
                        start=(kt == 0),
                        stop=(kt == KT - 1),
                    )
                nc.vector.tensor_copy(dst[:, nch * 512 : (nch + 1) * 512], ps[:])
                done.add(key)

            return work

        # V: out[n, c] = xT[:, ntile].T @ wv   -> [128 n, 256 c]
        def v_unit(nt):
            def work(cur_period):
                ps = sp_alloc([P, HL * DH], ("copy", cur_period))
                nch, sub = divmod(nt, NCH)
                for kt in range(KT):
                    nc.tensor.matmul(
                        ps[:],
                        xT[:, nch, kt, sub * P : (sub + 1) * P],
                        wv[:, kt, :],
                        start=(kt == 0),
                        stop=(kt == KT - 1),
                    )
                # scatter the 4 heads' 64 cols into the [NT, HL, 65] layout
                nc.vector.tensor_copy(
                    v[:, nt, :, 0:DH],
                    ps[:].rearrange("p (h d) -> p h d", h=HL),
                )
                done.add(("v", nt))

            return work

        # ones column of V (softmax denominator comes out of the PV matmul)
        nc.vector.memset(v[:, :, :, DH : DH + 1], 1.0)

        # PE warm-up: the tensor engine clock ramps with sustained use (full
        # speed only after ~3us of continuous execution).  While the xT DMA
        # streams in, run throwaway matmuls on a zeroed scratch region so the
        # real projections start at full clock instead of the 1.2 GHz
        # mid p-state.  ~64 x 128-col matmuls ~= 4-7us of filler.
        warm = sing.tile([P, 512], bf16, tag="warm")
        nc.vector.memset(warm[:], 0.0)

        def emit_warm(n):
            for _ in range(n):
                wps = sp_alloc([P, 512], ("warm", (-1, 0)))
                nc.tensor.matmul(
                    wps[:], warm[:, 0:P], warm[:], start=True, stop=True
                )

        emit_warm(16)


        # ---- attention ----
        # 8 blocks of 16 periods (one per (hp, ic)).  The exp stream paces the
        # kernel: ACT runs [128, 1024] exps back-to-back while the DVE handles
        # the S_DVE key blocks concurrently via the bit-hack.  PE emits scores
        # two periods ahead (spool rotation), weaves the remaining projection
        # work, and runs PV as dense bursts with quarter q3 deferred into the
        # next block so it never waits on a just-finished exp.
        blocks = [(hp, ic) for hp in range(2) for ic in range(NCH)]
        ats = {}
        opairs = {}
        sp_ahead = {}

        def emit_scores(b, jb):
            hp, ic = blocks[b]
            i0, j0 = ic * 512, jb * P
            assert ("k", hp, (jb * P) // 512) in done, ("k-chunk", b, jb)
            assert ("q", hp, ic) in done, ("q-chunk", b, jb)
            kind = "schr" if jb in _s_dve(b) else "exp"
            sp = sp_alloc([P, 1024], (kind, (b, jb)))
            nc.tensor.matmul(
                sp[:, 0:512],
                kT[hp][0:DH, j0 : j0 + P],
                qT[hp][0:DH, i0 : i0 + 512],
                start=True, stop=True, tile_position=(0, 0),
            )
            nc.tensor.matmul(
                sp[:, 512:1024],
                kT[hp][DH:P, j0 : j0 + P],
                qT[hp][DH:P, i0 : i0 + 512],
                start=True, stop=True, tile_position=(64, 0),
            )
            return sp

        def emit_exp(b, jb, sp):
            if jb in _s_dve(b):
                at = apool.tile([P, 1024], i16, tag="at", name="ats")
                nc.vector.tensor_scalar(
                    out=at[:],
                    in0=sp[:],
                    scalar1=SCH_A,
                    scalar2=SCH_B,
                    op0=mybir.AluOpType.mult,
                    op1=mybir.AluOpType.add,
                )
                ats[(b, jb)] = (at, True)
            else:
                at = apool.tile([P, 1024], bf16, tag="at", name="at")
                nc.scalar.activation(at[:], sp[:], mybir.ActivationFunctionType.Exp)
                ats[(b, jb)] = (at, False)

        def fetch_scores(b, jb):
            key = (b, jb)
            if key in sp_ahead:
                return sp_ahead.pop(key)
            return emit_scores(b, jb)

        def at_rhs(b, jb, col):
            t, is_i16 = ats[(b, jb)]
            rhs = t[:, 512 * col : 512 * col + 512]
            return rhs.bitcast(bf16) if is_i16 else rhs

        def emit_pv(b, jbs, last=False):
            """PV matmuls of block b for the given key blocks (dense burst)."""
            hp, ic = blocks[b]
            if b not in opairs:
                opairs[b] = opool.tile([DH + 1, 2, 512], f32, tag="oT", name="oT")
            oT = opairs[b]
            for jb in jbs:
                assert ("v", jb) in done, ("v-unit", b, jb)
            for col in range(2):
                for jb in jbs:
                    nc.tensor.matmul(
                        oT[:, col, :],
                        v[:, jb, 2 * hp + col, :],
                        at_rhs(b, jb, col),
                        start=(jb == 0), stop=(jb == NT - 1),
                    )
            for jb in jbs:
                del ats[(b, jb)]
            if last:
                i0 = ic * 512
                os = copool.tile([DH + 1, 2, 512], f32, tag="os", name="os")
                nc.vector.tensor_copy(os[:], oT[:])
                nc.sync.dma_start(
                    out=out_r[:, 2 * hp : 2 * hp + 2, i0 : i0 + 512],
                    in_=os[:],
                )

        # woven PE filler, scheduled by (block, period) but drained through a
        # pending queue gated by weave_safe(): projection chunks and V units
        # land (in emission order) before the first matmul that reads them.
        # Deadlines: q0cN by scores(block N) emission at (N-1, 14); v(4q..4q+3)
        # by PV(b0, q) at (1,8)/(1,12)/(2,0)/(2,4); kT[1]+q1c0 by the first
        # hp=1 scores emitted at (3, 14).
        weave = {}
        wsched = [
            ((0, 0), proj_unit(wk, kT[0], 0, 1, ("k", 0, 1))),
            ((0, 1), proj_unit(wk, kT[0], 0, 2, ("k", 0, 2))),
            ((0, 2), proj_unit(wk, kT[0], 0, 3, ("k", 0, 3))),
            ((0, 3), proj_unit(wq, qT[0], 0, 1, ("q", 0, 1))),
            ((0, 5), v_unit(0)),
            ((0, 7), v_unit(1)),
            ((0, 9), v_unit(2)),
            ((0, 11), v_unit(3)),
            ((1, 0), v_unit(4)),
            ((1, 2), v_unit(5)),
            ((1, 4), v_unit(6)),
            ((1, 6), v_unit(7)),
            ((1, 7), proj_unit(wq, qT[0], 0, 2, ("q", 0, 2))),
            ((1, 8), v_unit(8)),
            ((1, 10), v_unit(9)),
            ((1, 12), v_unit(10)),
            ((1, 14), v_unit(11)),
            ((2, 0), v_unit(12)),
            ((2, 1), v_unit(13)),
            ((2, 2), v_unit(14)),
            ((2, 3), v_unit(15)),
            ((2, 6), proj_unit(wq, qT[0], 0, 3, ("q", 0, 3))),
            ((0, 13), proj_unit(wk, kT[1], 1, 0, ("k", 1, 0))),
            ((3, 1), proj_unit(wk, kT[1], 1, 1, ("k", 1, 1))),
            ((3, 5), proj_unit(wk, kT[1], 1, 2, ("k", 1, 2))),
            ((4, 1), proj_unit(wk, kT[1], 1, 3, ("k", 1, 3))),
            ((3, 12), proj_unit(wq, qT[1], 1, 0, ("q", 1, 0))),
            ((4, 6), proj_unit(wq, qT[1], 1, 1, ("q", 1, 1))),
            ((5, 2), proj_unit(wq, qT[1], 1, 2, ("q", 1, 2))),
            ((5, 8), proj_unit(wq, qT[1], 1, 3, ("q", 1, 3))),
        ]
        for place, u in wsched:
            weave.setdefault(place, []).append(u)

        # PV quarter schedule: deferred ~1.5 blocks behind the exp stream so
        # early blocks aren't double-loaded (weave + PV), with a serialized
        # oT chain at the end (opool has a single buffer).
        pv_sched = {}

        def pv_at(period, src_b, jbs, last=False):
            pv_sched.setdefault(period, []).append((src_b, jbs, last))

        # q3 (with its DVE out-copy) is placed on periods clear of upcoming
        # DVE Schraudolph tiles: the copy would otherwise head-of-line block
        # the next schr in the DVE queue and stall the spool rotation.
        for b in range(4):
            pv_at((b + 1, 8), b, range(0, 4))
            pv_at((b + 1, 12), b, range(4, 8))
            pv_at((b + 2, 0), b, range(8, 12))
            pv_at((b + 2, 6), b, range(12, 16), last=True)
        pv_at((5, 8), 4, range(0, 4))
        pv_at((5, 12), 4, range(4, 8))
        pv_at((6, 0), 4, range(8, 12))
        pv_at((6, 5), 4, range(12, 16), last=True)
        pv_at((6, 7), 5, range(0, 4))
        pv_at((6, 9), 5, range(4, 8))
        pv_at((6, 11), 5, range(8, 12))
        pv_at((6, 14), 5, range(12, 16), last=True)
        pv_at((7, 0), 6, range(0, 4))
        pv_at((7, 1), 6, range(4, 8))
        pv_at((7, 2), 6, range(8, 12))
        pv_at((7, 3), 6, range(12, 16), last=True)
        pv_at((7, 5), 7, range(0, 4))
        pv_at((7, 8), 7, range(4, 8))
        pv_at((7, 12), 7, range(8, 12))
        pv_at((7, 14), 7, (12, 13))
        pv_at((7, 15), 7, (14, 15), last=True)

        # head: the k projection is split around more warm-up filler so the
        # PE stays ramped across the chunk-0 DMA tail instead of going idle.
        kps = sp_alloc([P, 512], ("copy", (-1, 0)))
        for kt in range(4):
            nc.tensor.matmul(
                kps[:], wk[:, kt, 0:P], xT[:, 0, kt, :],
                start=(kt == 0), stop=False,
            )
        emit_warm(8)
        for kt in range(4, KT):
            nc.tensor.matmul(
                kps[:], wk[:, kt, 0:P], xT[:, 0, kt, :],
                start=False, stop=(kt == KT - 1),
            )
        # ACT is idle this early and 'copy' shares exp's table set: doing the
        # k copy there lets it overlap the q projection + q copy on the DVE
        nc.scalar.copy(kT[0][:, 0:512], kps[:])
        done.add(("k", 0, 0))
        proj_unit(wq, qT[0], 0, 0, ("q", 0, 0))((-1, 1))
        LA = 2  # scores lookahead depth
        nblocks = len(blocks)
        for j in range(LA):
            sp_ahead[(0, j)] = emit_scores(0, j)
        pending = []
        for b in range(nblocks):
            for jb in range(NT):
                emit_exp(b, jb, fetch_scores(b, jb))
                # PV bursts first: they have no spool-slot waits, so the PE
                # makes progress while the upcoming la-scores' slot drains.
                for src_b, jbs, last in pv_sched.pop((b, jb), ()):
                    emit_pv(src_b, jbs, last=last)
                # woven PE filler (deadlock-gated; up to 2 units per period)
                pending.extend(weave.pop((b, jb), ()))
                emitted = 0
                while pending and emitted < 2 and weave_safe((b, jb)):
                    pending.pop(0)((b, jb))
                    emitted += 1
                la = jb + LA
                if la < NT:
                    if (b, la) not in sp_ahead:
                        sp_ahead[(b, la)] = emit_scores(b, la)
                elif b + 1 < nblocks:
                    sp_ahead[(b + 1, la - NT)] = emit_scores(b + 1, la - NT)
                if jb == NT - 1 and b + 1 < nblocks:
                    # boundary prefetch into the idle third spool slot: keeps
                    # the exp stream covered across the PV bursts
                    sp_ahead[(b + 1, LA)] = emit_scores(b + 1, LA)
        assert not pending and not weave and not pv_sched, (
            len(pending), sorted(weave), sorted(pv_sched)
        )

    nc.compile()
    return nc


def _get_nc():
    if "nc" not in _CACHE:
        _CACHE["nc"] = _build_nc()
    return _CACHE["nc"]


def _prepare_in_maps(x, w_qkv):
    bf = ml_dtypes.bfloat16
    x = np.asarray(x, dtype=np.float32)
    w = np.asarray(w_qkv, dtype=np.float32)
    scale = DH ** -0.5
    in_maps = []
    # xT n-chunk-major: element (p, nch, kt, n) = x[b].T[kt*128+p, nch*512+n]
    xT_b = [
        np.ascontiguousarray(
            x[b].T.reshape(KT, P, NCH, 512).transpose(1, 2, 0, 3).reshape(P, KT * N)
        ).astype(bf)
        for b in range(B)
    ]
    for c in range(8):
        b, hg = divmod(c, 4)
        cs = slice(hg * HL * DH, (hg + 1) * HL * DH)
        in_maps.append(
            {
                "xT": xT_b[b],
                "wq": np.ascontiguousarray(w[:, cs] * scale).astype(bf),
                "wk": np.ascontiguousarray(w[:, 1024:2048][:, cs]).astype(bf),
                "wv": np.ascontiguousarray(w[:, 2048:3072][:, cs]).astype(bf),
            }
        )
    return in_maps


def _assemble(outs):
    full = np.empty((B, N, HEADS * DH), dtype=np.float32)
    for c in range(8):
        b, hg = divmod(c, 4)
        o = outs[c].reshape(HL, DH + 1, N)
        norm = o[:, :DH, :] / o[:, DH : DH + 1, :]  # [hl, d, n]
        full[b, :, hg * HL * DH : (hg + 1) * HL * DH] = norm.transpose(2, 0, 1).reshape(
            N, HL * DH
        )
    return full


def kernel(x, w_qkv):
    global LAST_RESULTS
    from concourse.bass_utils import run_bass_kernel_spmd

    nc = _get_nc()
    in_maps = _prepare_in_maps(x, w_qkv)
    last_err = None
    for _ in range(3):  # the runtime occasionally throws a transient device error
        try:
            res = run_bass_kernel_spmd(
                nc,
                in_maps,
                core_ids=list(range(8)),
                trace=TRACE,
                trace_cores=[0] if TRACE else None,
            )
            break
        except Exception as e:
            last_err = e
    else:
        raise last_err
    LAST_RESULTS = res
    return _assemble([r["out"] for r in res.results])
